# revision 15
# baseline (speedup 1.0000x reference)
"""Trainium2 Bass kernel for nn_Autoencoder_10874857193743 (vq_codebook).

Sharding: data-parallel over batch B=16 across 8 NeuronCores (B_local=2).
Device layout is H-major (hidden dim on partitions) end-to-end; the LSTM runs
weight-stationary (gatesT tiles = W_block.T @ h_cols) so gates/states/eltwise
all sit on partitions 0-127 with batch on the small free dim.
"""
import numpy as np

B, T, S, H, K, NQ = 16, 512, 320, 512, 1024, 4
NCORES = 8
BL = B // NCORES          # batch per core (2)
TOK = BL * T              # tokens per core (1024)
HC = H // 128             # H chunks (4)
KC = K // 128             # codebook chunks (8)
NM = 4 * H // 128         # gate m-tiles (16)
TT = TOK // 128           # token tiles per core (8)

# m-tile order: [i(0:4), f(4:8), o(8:12), g(12:16)] (sigmoid gates contiguous)
GATE_PERM = [0, 1, 2, 3, 4, 5, 6, 7, 12, 13, 14, 15, 8, 9, 10, 11]

_CACHE = {}


def _build(t_steps, debug=False):
    import concourse.bacc as bacc
    import concourse.tile as tile
    import concourse.mybir as mybir

    f32 = mybir.dt.float32
    bf16 = mybir.dt.bfloat16
    u32 = mybir.dt.uint32
    AF = mybir.ActivationFunctionType
    ALU = mybir.AluOpType

    nc = bacc.Bacc()
    Pm = lambda name, shape, dt: nc.declare_dram_parameter(name, list(shape), dt, isOutput=False)
    Om = lambda name, shape, dt: nc.declare_dram_parameter(name, list(shape), dt, isOutput=True)

    waveT = Pm("waveT", (128, 3, TOK), f32)
    encw = Pm("encw", (128, 3, HC, 128), f32)
    encb = Pm("encb", (1, H), f32)
    lng = Pm("lng", (128, HC), f32)
    lnb = Pm("lnb", (128, HC), f32)
    e2t = Pm("e2t", (128, NQ, HC, K), f32)
    esqn = Pm("esqn", (1, NQ, K), f32)
    erec = Pm("erec", (128, NQ, KC, HC, 128), f32)
    kio = Pm("kio", (128, KC), f32)
    wi1b = Pm("wi1b", (128, HC, NM, 128), f32)
    b1r = Pm("b1r", (1, 4 * H), f32)
    wh1b = Pm("wh1b", (128, HC, NM, 128), f32)
    w2b = Pm("w2b", (128, 2 * HC, NM, 128), f32)
    b2t = Pm("b2t", (128, NM, BL), f32)
    decw = Pm("decw", (128, HC, S), f32)
    decb = Pm("decb", (1, S), f32)
    onesr = Pm("onesr", (1, 128), f32)
    ones512f = Pm("ones512f", (1, 512), f32)
    ones512b = Pm("ones512b", (1, 512), bf16)
    onescol = Pm("onescol", (128, 1), f32)
    onescolb = Pm("onescolb", (128, 1), bf16)
    i128 = Pm("i128", (128, 128), f32)

    recon = Om("recon", (BL, T, S), f32)
    losses = Om("losses", (1, NQ * HC), f32)
    dbg = {}
    if debug:
        dbg["r0"] = Om("dbg_r0", (128, HC, TOK), f32)
        dbg["qs"] = Om("dbg_qs", (128, HC, TOK), f32)
        dbg["idx"] = Om("dbg_idx", (NQ, 1, TOK), f32)
        dbg["xp1"] = Om("dbg_xp1", (128, NM, BL, T), f32)
        dbg["h1"] = Om("dbg_h1", (128, HC, BL, T), f32)
        dbg["h2"] = Om("dbg_h2", (128, HC, BL, T), f32)

    ctx_lp = nc.allow_low_precision("bf16 activations/gates pipeline")
    ctx_lp.__enter__()
    with tile.TileContext(nc) as tc:
        cpool = tc.alloc_tile_pool(name="consts", bufs=1)
        t_onesr = cpool.tile([1, 128], f32)
        nc.sync.dma_start(t_onesr[:], onesr[:])
        t_ones512f = cpool.tile([1, 512], f32)
        nc.sync.dma_start(t_ones512f[:], ones512f[:])
        t_ones512b = cpool.tile([1, 512], bf16)
        nc.sync.dma_start(t_ones512b[:], ones512b[:])
        t_onescol = cpool.tile([128, 1], f32)
        nc.sync.dma_start(t_onescol[:], onescol[:])
        t_onescolb = cpool.tile([128, 1], bf16)
        nc.sync.dma_start(t_onescolb[:], onescolb[:])
        t_i128 = cpool.tile([128, 128], f32)
        nc.sync.dma_start(t_i128[:], i128[:])
        tkio = cpool.tile([128, KC], f32)
        nc.sync.dma_start(tkio[:], kio[:])

        xpool = tc.alloc_tile_pool(name="lstmstate", bufs=1)
        xp1 = xpool.tile([128, NM, BL, T], f32)
        h1a = xpool.tile([128, HC, BL, T], f32)
        h2a = xpool.tile([128, HC, BL, T], f32)
        spool = tc.alloc_tile_pool(name="state", bufs=1)
        qsT = spool.tile([128, HC, TOK], f32)
        lacc = spool.tile([128, NQ * HC], f32)
        rT = spool.tile([128, HC, TOK], f32)

        # ================= Phase E: encoder + LN + ReLU =================
        with tc.tile_pool(name="enc", bufs=1) as epool, \
             tc.tile_pool(name="encp", bufs=2, space="PSUM") as eppool:
            wv = epool.tile([128, 3, TOK], f32)
            nc.sync.dma_start(wv[:], waveT[:])
            ew = epool.tile([128, 3, HC, 128], f32)
            nc.sync.dma_start(ew[:], encw[:])
            eb = epool.tile([1, H], f32)
            nc.sync.dma_start(eb[:], encb[:])
            tg_ = epool.tile([128, HC], f32)
            nc.sync.dma_start(tg_[:], lng[:])
            tb_ = epool.tile([128, HC], f32)
            nc.sync.dma_start(tb_[:], lnb[:])

            xT = epool.tile([128, HC, TOK], f32)
            for ht in range(HC):
                for ns in range(2):
                    px = eppool.tile([128, 512], f32, tag="px")
                    nc.tensor.matmul(px[:], eb[0:1, 128 * ht:128 * (ht + 1)], t_ones512f[:],
                                     start=True, stop=False)
                    for sc in range(3):
                        nc.tensor.matmul(px[:], ew[:, sc, ht, :], wv[:, sc, 512 * ns:512 * (ns + 1)],
                                         start=False, stop=(sc == 2))
                    nc.scalar.activation(xT[:, ht, 512 * ns:512 * (ns + 1)], px[:], AF.Copy)

            pm = eppool.tile([1, TOK], f32, tag="lnp")
            for ns in range(2):
                for c in range(HC):
                    nc.tensor.matmul(pm[0:1, 512 * ns:512 * (ns + 1)], t_onescol[:],
                                     xT[:, c, 512 * ns:512 * (ns + 1)],
                                     start=(c == 0), stop=(c == HC - 1))
            mu = epool.tile([1, TOK], f32, tag="ln1", bufs=2)
            nc.scalar.activation(mu[:], pm[:], AF.Copy, scale=1.0 / H)
            pmb = eppool.tile([128, TOK], f32, tag="lnp")
            for ns in range(2):
                nc.tensor.matmul(pmb[:, 512 * ns:512 * (ns + 1)], t_onesr[:],
                                 mu[0:1, 512 * ns:512 * (ns + 1)], start=True, stop=True)
            for c in range(HC):
                nc.vector.tensor_tensor(out=xT[:, c, :], in0=xT[:, c, :], in1=pmb[:], op=ALU.subtract)
            ps = eppool.tile([1, TOK], f32, tag="lnp")
            for ns in range(2):
                for c in range(HC):
                    sq = epool.tile([128, 512], f32, tag="sq")
                    nc.scalar.activation(sq[:], xT[:, c, 512 * ns:512 * (ns + 1)], AF.Square)
                    nc.tensor.matmul(ps[0:1, 512 * ns:512 * (ns + 1)], t_onescol[:], sq[:],
                                     start=(c == 0), stop=(c == HC - 1))
            var = epool.tile([1, TOK], f32, tag="ln1", bufs=2)
            nc.scalar.activation(var[:], ps[:], AF.Copy, scale=1.0 / H)
            teps = epool.tile([1, 1], f32)
            nc.vector.memset(teps[:], 1e-5)
            sd = epool.tile([1, TOK], f32, tag="ln1", bufs=2)
            nc.scalar.activation(sd[:], var[:], AF.Sqrt, bias=teps[:])
            rstd = epool.tile([1, TOK], f32, tag="ln1", bufs=2)
            nc.vector.reciprocal(rstd[:], sd[:])
            prb = eppool.tile([128, TOK], f32, tag="lnp")
            for ns in range(2):
                nc.tensor.matmul(prb[:, 512 * ns:512 * (ns + 1)], t_onesr[:],
                                 rstd[0:1, 512 * ns:512 * (ns + 1)], start=True, stop=True)
            for c in range(HC):
                nc.vector.tensor_tensor(out=xT[:, c, :], in0=xT[:, c, :], in1=prb[:], op=ALU.mult)
                nc.scalar.activation(rT[:, c, :], xT[:, c, :], AF.Relu,
                                     scale=tg_[:, c:c + 1], bias=tb_[:, c:c + 1])
            nc.vector.memset(qsT[:], 0.0)
            nc.vector.memset(lacc[:], 0.0)
            if debug:
                nc.sync.dma_start(dbg["r0"][:], rT[:])

        # ================= Phase V: residual VQ =================
        with tc.tile_pool(name="vq", bufs=1) as vpool, \
             tc.tile_pool(name="vqp", bufs=2, space="PSUM") as vppool, \
             tc.tile_pool(name="vqp1", bufs=1, space="PSUM") as vppool1:
            for q in range(NQ):
                te2 = vpool.tile([128, HC, K], f32, tag="te2")
                nc.sync.dma_start(te2[:], e2t[:, q, :, :])
                ter = vpool.tile([128, KC, HC, 128], f32, tag="ter")
                nc.sync.dma_start(ter[:], erec[:, q, :, :, :])
                tesq = vpool.tile([1, K], f32, tag="rowbuf")
                nc.sync.dma_start(tesq[:], esqn[0:1, q, :])

                pidx = vppool1.tile([1, TOK], f32, tag="pidx")
                for tt in range(TT):
                    sp = vppool.tile([128, K], f32, tag="vps")
                    for ns in range(2):
                        for c in range(HC):
                            nc.tensor.matmul(sp[:, 512 * ns:512 * (ns + 1)],
                                             rT[:, c, 128 * tt:128 * (tt + 1)],
                                             te2[:, c, 512 * ns:512 * (ns + 1)],
                                             start=(c == 0), stop=False)
                        nc.tensor.matmul(sp[:, 512 * ns:512 * (ns + 1)], t_onesr[:],
                                         tesq[0:1, 512 * ns:512 * (ns + 1)], start=False, stop=True)
                    ssb = vpool.tile([128, K], f32, tag="ssb")
                    nc.vector.tensor_copy(ssb[:], sp[:])
                    mx = vpool.tile([128, 8], f32, tag="mx")
                    mi = vpool.tile([128, 8], u32, tag="mi")
                    nc.vector.max_with_indices(mx[:], mi[:], ssb[:])
                    idxc = vpool.tile([128, 1], f32, tag="idxc")
                    nc.vector.tensor_copy(idxc[:], mi[:, 0:1])
                    nc.tensor.transpose(pidx[0:1, 128 * tt:128 * (tt + 1)], idxc[:], t_i128[:])
                idxrow = vpool.tile([1, TOK], f32, tag="rowbuf")
                nc.scalar.activation(idxrow[:], pidx[:], AF.Copy)
                if debug:
                    nc.sync.dma_start(dbg["idx"][q, 0:1, :], idxrow[:])
                pib = vppool.tile([128, TOK], f32, tag="vps")
                for ns in range(2):
                    nc.tensor.matmul(pib[:, 512 * ns:512 * (ns + 1)], t_onesr[:],
                                     idxrow[0:1, 512 * ns:512 * (ns + 1)], start=True, stop=True)
                mask = vpool.tile([128, KC, TOK], f32, tag="mask")
                for kc in range(KC):
                    nc.vector.tensor_scalar(out=mask[:, kc, :], in0=pib[:],
                                            scalar1=tkio[:, kc:kc + 1], scalar2=None,
                                            op0=ALU.is_equal)
                for ht in range(HC):
                    for ns in range(2):
                        qp = vppool.tile([128, 512], f32, tag="qp")
                        for kc in range(KC):
                            nc.tensor.matmul(qp[:], ter[:, kc, ht, :],
                                             mask[:, kc, 512 * ns:512 * (ns + 1)],
                                             start=(kc == 0), stop=(kc == KC - 1))
                        sl = slice(512 * ns, 512 * (ns + 1))
                        nc.vector.tensor_tensor(out=qsT[:, ht, sl], in0=qsT[:, ht, sl],
                                                in1=qp[:], op=ALU.add)
                        nc.vector.tensor_tensor(out=rT[:, ht, sl], in0=rT[:, ht, sl],
                                                in1=qp[:], op=ALU.subtract)
                scr = vpool.tile([128, TOK], bf16, tag="ssb")
                for c in range(HC):
                    nc.scalar.activation(scr[:], rT[:, c, :], AF.Square,
                                         accum_out=lacc[:, HC * q + c:HC * q + c + 1])
            pl = vppool.tile([1, NQ * HC], f32, tag="qp")
            nc.tensor.matmul(pl[:], t_onescol[:], lacc[:], start=True, stop=True)
            lsb = vpool.tile([1, NQ * HC], f32, tag="lsb")
            nc.scalar.activation(lsb[:], pl[:], AF.Copy)
            nc.sync.dma_start(losses[:], lsb[:])
            if debug:
                nc.sync.dma_start(dbg["qs"][:], qsT[:])

        # ================= Phase P: x1proj = qsum @ Wi1^T + b1 =================
        with tc.tile_pool(name="pp", bufs=1) as ppool, \
             tc.tile_pool(name="ppp", bufs=2, space="PSUM") as pppool:
            twi = ppool.tile([128, HC, NM, 128], f32)
            nc.sync.dma_start(twi[:], wi1b[:])
            tb1 = ppool.tile([1, 4 * H], f32)
            nc.sync.dma_start(tb1[:], b1r[:])
            for m in range(NM):
                for nb in range(BL):
                    pxp = pppool.tile([128, 512], f32, tag="pxp")
                    nc.tensor.matmul(pxp[:], tb1[0:1, 128 * m:128 * (m + 1)], t_ones512f[:],
                                     start=True, stop=False)
                    for c in range(HC):
                        nc.tensor.matmul(pxp[:], twi[:, c, m, :],
                                         qsT[:, c, 512 * nb:512 * (nb + 1)],
                                         start=False, stop=(c == HC - 1))
                    nc.scalar.activation(xp1[:, m, nb, 0:T], pxp[:, 0:T], AF.Copy)
            if debug:
                nc.sync.dma_start(dbg["xp1"][:], xp1[:])
        spool.release()

        # ================= Phase L: 2-layer LSTM =================
        with tc.tile_pool(name="lw", bufs=1) as lwpool, \
             tc.tile_pool(name="ls", bufs=3) as lspool, \
             tc.tile_pool(name="lsp", bufs=2, space="PSUM") as lppool:
            twh1 = lwpool.tile([128, HC, NM, 128], f32)
            nc.sync.dma_start(twh1[:], wh1b[:])
            tw2 = lwpool.tile([128, 2 * HC, NM, 128], f32)
            nc.sync.dma_start(tw2[:], w2b[:])
            tb2 = lwpool.tile([128, NM, BL], f32)
            nc.sync.dma_start(tb2[:], b2t[:])
            c1 = lwpool.tile([128, HC, BL], f32)
            c2 = lwpool.tile([128, HC, BL], f32)
            nc.vector.memset(c1[:], 0.0)
            nc.vector.memset(c2[:], 0.0)

            def cell(layer, t):
                if layer == 1:
                    g = lspool.tile([128, NM, BL], f32, tag="g1")
                    if t > 0:
                        pg = lppool.tile([128, NM, BL], f32, tag="pg1")
                        for m in range(NM):
                            for c in range(HC):
                                nc.tensor.matmul(pg[:, m, :], twh1[:, c, m, :],
                                                 h1a[:, c, :, t - 1],
                                                 start=(c == 0), stop=(c == HC - 1))
                        nc.vector.tensor_tensor(out=g[:], in0=pg[:], in1=xp1[:, :, :, t], op=ALU.add)
                    else:
                        nc.vector.tensor_copy(g[:], xp1[:, :, :, t])
                    cst, ha = c1, h1a
                else:
                    pg = lppool.tile([128, NM, BL], f32, tag="pg2")
                    nch = 2 * HC if t > 0 else HC
                    for m in range(NM):
                        for c in range(nch):
                            rhs = h1a[:, c, :, t] if c < HC else h2a[:, c - HC, :, t - 1]
                            nc.tensor.matmul(pg[:, m, :], tw2[:, c, m, :], rhs,
                                             start=(c == 0), stop=(c == nch - 1))
                    g = lspool.tile([128, NM, BL], f32, tag="g2")
                    nc.vector.tensor_tensor(out=g[:], in0=pg[:], in1=tb2[:], op=ALU.add)
                    cst, ha = c2, h2a
                a = lspool.tile([128, NM, BL], f32, tag=f"a{layer}")
                nc.scalar.activation(a[:, 0:12, :], g[:, 0:12, :], AF.Sigmoid)
                nc.scalar.activation(a[:, 12:16, :], g[:, 12:16, :], AF.Tanh)
                t1 = lspool.tile([128, HC, BL], f32, tag=f"t1_{layer}")
                nc.vector.tensor_tensor(out=t1[:], in0=a[:, 4:8, :], in1=cst[:], op=ALU.mult)
                t2 = lspool.tile([128, HC, BL], f32, tag=f"t2_{layer}")
                nc.vector.tensor_tensor(out=t2[:], in0=a[:, 0:4, :], in1=a[:, 12:16, :], op=ALU.mult)
                nc.vector.tensor_tensor(out=cst[:], in0=t1[:], in1=t2[:], op=ALU.add)
                tc_ = lspool.tile([128, HC, BL], f32, tag=f"tc{layer}")
                nc.scalar.activation(tc_[:], cst[:], AF.Tanh)
                nc.vector.tensor_tensor(out=ha[:, :, :, t], in0=a[:, 8:12, :], in1=tc_[:], op=ALU.mult)

            for t in range(t_steps):
                cell(1, t)
                cell(2, t)
            if debug:
                nc.sync.dma_start(dbg["h1"][:], h1a[:])
                nc.sync.dma_start(dbg["h2"][:], h2a[:])

        # ================= Phase D: decoder =================
        with tc.tile_pool(name="dp", bufs=2) as dpool, \
             tc.tile_pool(name="dpp", bufs=2, space="PSUM") as dppool:
            tdw = dpool.tile([128, HC, S], f32, tag="tdw")
            nc.sync.dma_start(tdw[:], decw[:])
            tdb = dpool.tile([1, S], f32, tag="tdb")
            nc.sync.dma_start(tdb[:], decb[:])
            span = min(128, t_steps)
            for nb in range(BL):
                for tt2 in range(max(1, t_steps // 128)):
                    pd = dppool.tile([128, S], f32, tag="pd")
                    nc.tensor.matmul(pd[0:span, :], t_onesr[0:1, 0:span], tdb[:],
                                     start=True, stop=False)
                    for c in range(HC):
                        nc.tensor.matmul(pd[0:span, :],
                                         h2a[:, c, nb, 128 * tt2:128 * tt2 + span],
                                         tdw[:, c, :], start=False, stop=(c == HC - 1))
                    osb = dpool.tile([128, S], f32, tag="osb")
                    nc.scalar.activation(osb[0:span, :], pd[0:span, :], AF.Copy)
                    nc.sync.dma_start(recon[nb, 128 * tt2:128 * tt2 + span, :], osb[0:span, :])

        xpool.release()
        cpool.release()
    ctx_lp.__exit__(None, None, None)

    nc.compile()
    return nc


def _preprocess(inputs):
    """Host-side: shard batch + reorganize weights into device layouts."""
    f32 = np.float32
    import ml_dtypes
    bf16 = ml_dtypes.bfloat16
    wave = np.asarray(inputs["waveform"], f32)
    enc_w = np.asarray(inputs["enc_w"], f32)
    enc_b = np.asarray(inputs["enc_b"], f32)
    ln_g = np.asarray(inputs["ln_g"], f32)
    ln_b = np.asarray(inputs["ln_b"], f32)
    E = np.asarray(inputs["codebooks"], f32)
    wi = np.asarray(inputs["lstm_wi"], f32)
    wh = np.asarray(inputs["lstm_wh"], f32)
    bi = np.asarray(inputs["lstm_bi"], f32)
    bh = np.asarray(inputs["lstm_bh"], f32)
    dec_w = np.asarray(inputs["dec_w"], f32)
    dec_b = np.asarray(inputs["dec_b"], f32)

    shared = {}
    ewp = np.zeros((384, H), f32)
    ewp[:S] = enc_w
    shared["encw"] = np.ascontiguousarray(ewp.reshape(3, 128, HC, 128).transpose(1, 0, 2, 3))
    shared["encb"] = enc_b.reshape(1, H).copy()
    shared["lng"] = np.ascontiguousarray(ln_g.reshape(HC, 128).T)
    shared["lnb"] = np.ascontiguousarray(ln_b.reshape(HC, 128).T)
    e2 = 2.0 * E.transpose(0, 2, 1)
    shared["e2t"] = np.ascontiguousarray(e2.reshape(NQ, HC, 128, K).transpose(2, 0, 1, 3))
    shared["esqn"] = np.ascontiguousarray((-(E ** 2).sum(-1)).reshape(1, NQ, K)).astype(f32)
    shared["erec"] = np.ascontiguousarray(E.reshape(NQ, KC, 128, HC, 128).transpose(2, 0, 1, 3, 4))
    shared["kio"] = (np.arange(128)[:, None] + 128.0 * np.arange(KC)[None, :]).astype(f32)

    perm = np.concatenate([np.arange(128 * m, 128 * (m + 1)) for m in GATE_PERM])

    def wblocks(wmat_t, nchunk, dt=bf16):
        wp = wmat_t[:, perm]
        return np.ascontiguousarray(wp.reshape(nchunk, 128, NM, 128).transpose(1, 0, 2, 3)).astype(dt)

    shared["wi1b"] = wblocks(wi[0].T, HC, f32)
    shared["wh1b"] = wblocks(wh[0].T, HC, f32)
    shared["w2b"] = wblocks(np.concatenate([wi[1].T, wh[1].T], axis=0), 2 * HC, f32)
    shared["b1r"] = (bi[0] + bh[0])[perm].reshape(1, 4 * H).astype(f32)
    b2 = (bi[1] + bh[1])[perm]
    shared["b2t"] = np.ascontiguousarray(
        np.repeat(b2.reshape(NM, 128).T[:, :, None], BL, axis=2)).astype(f32)
    shared["decw"] = np.ascontiguousarray(dec_w.reshape(HC, 128, S).transpose(1, 0, 2))
    shared["decb"] = dec_b.reshape(1, S).copy()
    shared["onesr"] = np.ones((1, 128), f32)
    shared["ones512f"] = np.ones((1, 512), f32)
    shared["ones512b"] = np.ones((1, 512), bf16)
    shared["onescol"] = np.ones((128, 1), f32)
    shared["onescolb"] = np.ones((128, 1), bf16)
    shared["i128"] = np.eye(128, dtype=f32)

    in_maps = []
    for core in range(NCORES):
        m = dict(shared)
        wslice = wave[BL * core:BL * (core + 1)].reshape(BL, T, S)
        wpad = np.zeros((BL, T, 384), f32)
        wpad[:, :, :S] = wslice
        m["waveT"] = np.ascontiguousarray(wpad.reshape(TOK, 3, 128).transpose(2, 1, 0))
        in_maps.append(m)
    return in_maps


def _run(inputs, t_steps=T, debug=False):
    from concourse.bass_utils import run_bass_kernel_spmd

    key = (t_steps, debug)
    if key not in _CACHE:
        _CACHE[key] = _build(t_steps, debug=debug)
    nc = _CACHE[key]
    in_maps = _preprocess(inputs)
    return run_bass_kernel_spmd(nc, in_maps, list(range(NCORES))).results


def kernel(**inputs):
    res = _run(inputs, T, False)
    recon = np.zeros((B, T * S), np.float32)
    loss = np.zeros(NQ, np.float64)
    for core in range(NCORES):
        r = res[core]
        recon[BL * core:BL * (core + 1)] = r["recon"].reshape(BL, T * S)
        loss += r["losses"].reshape(NQ, HC).sum(1)
    loss = (loss / (B * T * H)).astype(np.float32)
    return recon, loss
